# revision 11
# baseline (speedup 1.0000x reference)
"""Trainium2 Bass kernel for DualGraphConvolution.

reference math (N=8192, D=512):
    node_att = softmax(x @ node_w, axis=0)            # [N, 1]
    h        = x @ edge_w                             # [N, D]
    e        = h @ h.T ; masked where adj <= 0        # [N, N]
    edge_att = softmax(e, axis=1)                     # [N, N]
    out      = (adj * node_att * edge_att) @ (x @ weight) + bias

Distribution: row-shard the N dimension over 8 NeuronCores (1024 rows each).
Each core receives a *rotated* copy of the inputs (its own rows first) so the
SPMD program only ever uses static indices; column order of the j-contraction
is irrelevant because it is summed over.

Per core the kernel computes, for its 1024 rows r:
    m[r, j] = e[r, j] + (adj[r, j] - 1) * 1000        # masked-out cols pushed
                                                      # ~-1000 below the row max
    rowmax, t = exp(m - rowmax), Z = sum t            # online softmax over 4
                                                      # column sweeps of 2048
    O[r, :] = t @ support                             # support = x @ weight
    out = O * (exp(p_r) / (sum_k exp(p_k)) / Z) + bias  # p = x @ node_w

Matmul operands are fp16 (PE runs them at full rate; ~1e-3 relative error),
all accumulation/softmax state is fp32.
"""

import numpy as np

import concourse.bass as bass
import concourse.mybir as mybir
import concourse.tile as tile
from concourse import bacc
from concourse.bass_utils import run_bass_kernel_spmd

F16 = mybir.dt.float16
F32 = mybir.dt.float32
U8 = mybir.dt.uint8
ALU = mybir.AluOpType
ACTF = mybir.ActivationFunctionType
AX = mybir.AxisListType

N = 8192
D = 512
NCORES = 8
JSWEEP = 1024  # columns per online-softmax sweep
NEG_INIT = -3.0e38
MASK_SHIFT = 1000.0  # adj==0 columns get e - 1000: far below row max, exp -> 0


def build_program(n=N, d=D, ncores=NCORES, jsweep=JSWEEP,
                  dbg_skip_main=False, dbg_skip_nodestats=False,
                  dbg_main_upto=None, body_reps=1):
    loc = n // ncores          # rows owned by this core
    rb = loc // 128            # 128-row blocks per core
    kc = d // 128              # contraction chunks of 128
    nsweep = n // jsweep       # online-softmax sweeps
    jt = jsweep // 512         # 512-col j tiles per sweep
    jc = jsweep // 128         # 128-col j chunks per sweep
    rchunks = n // 128
    gw = 8 if jc % 8 == 0 else 4
    assert jc % gw == 0

    nc = bacc.Bacc("TRN2", target_bir_lowering=False, debug=False,
                   num_devices=ncores)

    xt_d = nc.dram_tensor("xt", [kc, 128, n], F16, kind="ExternalInput")
    adjp_d = nc.dram_tensor("adjp", [loc, n // 8], U8, kind="ExternalInput")
    ew_d = nc.dram_tensor("ew", [kc, 128, d], F16, kind="ExternalInput")
    wt_d = nc.dram_tensor("wt", [kc, 128, d], F16, kind="ExternalInput")
    nw_d = nc.dram_tensor("nw", [kc, 128, 1], F16, kind="ExternalInput")
    bias_d = nc.dram_tensor("biasr", [1, d], F32, kind="ExternalInput")
    id_d = nc.dram_tensor("ident", [128, 128], F16, kind="ExternalInput")
    out_d = nc.dram_tensor("out", [loc, d], F16, kind="ExternalOutput")

    with tile.TileContext(nc) as tc:
        for _rep in range(body_reps):
            with (
                tc.tile_pool(name="const", bufs=1) as constp,
                tc.tile_pool(name="big", bufs=1) as bigp,
                tc.tile_pool(name="vec", bufs=6) as vecp,
            ):
                ew_sb = constp.tile([128, kc, d], F16)
                nc.sync.dma_start(ew_sb[:], ew_d.rearrange("c p f -> p c f"))
                wt_sb = constp.tile([128, kc, d], F16)
                nc.sync.dma_start(wt_sb[:], wt_d.rearrange("c p f -> p c f"))
                nw_sb = constp.tile([128, kc, 1], F16)
                nc.sync.dma_start(nw_sb[:], nw_d.rearrange("c p f -> p c f"))
                bias1_sb = constp.tile([1, d], F32)
                nc.sync.dma_start(bias1_sb[:], bias_d[:])
                id_sb = constp.tile([128, 128], F16)
                nc.sync.dma_start(id_sb[:], id_d[:])
                adjp_sb = constp.tile([128, rb, n // 8], U8)
                nc.sync.dma_start(
                    adjp_sb[:], adjp_d.rearrange("(b p) v -> p b v", p=128))

                hT_sb = bigp.tile([128, kc, n], F16)   # h[r, dd] at [dd%128, dd//128, r]
                sup_sb = bigp.tile([128, rchunks, d], F16)  # support[rc*128+p, f]
                pzp = bigp.tile([1, n // 512], F32)    # per-r-tile sums of exp(p)
                ploc_sb = bigp.tile([128, rb], F32)    # exp(p) for local rows
                ones_row = constp.tile([1, 128], F32)
                nc.vector.memset(ones_row[:], 1.0)
                idk_sb = constp.tile([128, 128], F16)
                nc.vector.tensor_scalar_mul(idk_sb[:], id_sb[:], MASK_SHIFT)

                # ---- phase 0 (replicated): hT, support, p for all rows ----
                xt_view = xt_d.rearrange("c p r -> p c r")
                with (
                    tc.tile_pool(name="ph0", bufs=3) as ph0p,
                    tc.tile_pool(name="ph0ps", bufs=2, space="PSUM") as ph0ps,
                    tc.tile_pool(name="ph0ps1", bufs=1, space="PSUM") as ph0ps1,
                ):
                    for rt in range(n // 512):
                        xt_t = ph0p.tile([128, kc, 512], F16, tag="xt")
                        nc.sync.dma_start(
                            xt_t[:], xt_view[:, :, rt * 512:(rt + 1) * 512])
                        for dcp in range(kc // 2):
                            # two 512-wide groups into one 2-bank psum tile,
                            # one strided copy writes both hT d-chunks
                            hps = ph0ps.tile([128, 1024], F32, tag="hps")
                            for half in range(2):
                                dc = dcp * 2 + half
                                for c in range(kc):
                                    nc.tensor.matmul(
                                        hps[:, half * 512:(half + 1) * 512],
                                        ew_sb[:, c, dc * 128:(dc + 1) * 128],
                                        xt_t[:, c, :], start=(c == 0),
                                        stop=(c == kc - 1))
                            nc.vector.tensor_copy(
                                out=hT_sb[:, dcp * 2:dcp * 2 + 2,
                                          rt * 512:(rt + 1) * 512],
                                in_=hps[:].rearrange("p (h f) -> p h f", h=2))
                        for rs in range(4):
                            rch = rt * 4 + rs
                            sps = ph0ps.tile([128, d], F32, tag="sps")
                            for c in range(kc):
                                nc.tensor.matmul(
                                    sps[:], xt_t[:, c, rs * 128:(rs + 1) * 128],
                                    wt_sb[:, c, :], start=(c == 0), stop=(c == kc - 1))
                            nc.vector.tensor_copy(out=sup_sb[:, rch, :],
                                                  in_=sps[:])
                        # p slice [1, 512] via nw as the 1-col stationary
                        pps = ph0ps1.tile([1, 512], F32, tag="pps")
                        for c in range(kc):
                            nc.tensor.matmul(
                                pps[:], nw_sb[:, c, :], xt_t[:, c, :],
                                start=(c == 0), stop=(c == kc - 1))
                        pe_t = ph0p.tile([1, 512], F32, tag="pe")
                        nc.scalar.activation(pe_t[:], pps[:], ACTF.Exp,
                                             accum_out=pzp[:, rt:rt + 1])
                        if rt * 512 < loc:
                            # local rows: spread exp(p) across partitions via
                            # K=1 matmuls with the [1,128] slice stationary
                            for i in range(4):
                                b0 = rt * 4 + i
                                if b0 >= rb:
                                    break
                                tp_ps = ph0ps1.tile([128, 1], F32, tag="ptp")
                                nc.tensor.matmul(
                                    tp_ps[:], pe_t[:, i * 128:(i + 1) * 128],
                                    ones_row[:, 0:1])
                                nc.vector.tensor_copy(
                                    out=ploc_sb[:, b0:b0 + 1], in_=tp_ps[:])

                # ---- bias broadcast [1,d] -> [128,d] via K=1 matmul ----
                bias_sb = bigp.tile([128, d], F32)
                with tc.tile_pool(name="bbr", bufs=1, space="PSUM") as bbp:
                    bps = bbp.tile([128, d], F32, tag="bps")
                    nc.tensor.matmul(bps[:], ones_row[:], bias1_sb[:])
                    nc.vector.tensor_copy(out=bias_sb[:], in_=bps[:])

                # ---- node attention: scale0 = exp(p_loc) / sum(exp(p)) ----
                scale0 = bigp.tile([128, rb], F32)
                if dbg_skip_nodestats:
                    nc.vector.memset(scale0[:], 1.0)
                else:
                    pz = vecp.tile([1, 1], F32, tag="pz")
                    nc.vector.reduce_sum(pz[:], pzp[:], axis=AX.X)
                    pzi = vecp.tile([1, 1], F32, tag="pzi")
                    nc.vector.reciprocal(pzi[:], pz[:])
                    with tc.tile_pool(name="nps", bufs=2, space="PSUM") as npsp:
                        # broadcast 1/pz to all partitions via K=1 matmul
                        pzb_ps = npsp.tile([128, 1], F32, tag="pzbps")
                        nc.tensor.matmul(pzb_ps[:], ones_row[:], pzi[:])
                        pzb = vecp.tile([128, 1], F32, tag="pzb")
                        nc.vector.tensor_copy(out=pzb[:], in_=pzb_ps[:])
                    nc.vector.tensor_scalar_mul(scale0[:], ploc_sb[:], pzb[:])

                if dbg_skip_main:
                    with tc.tile_pool(name="dbgo", bufs=2) as dbgo:
                        for b in range(rb):
                            o_t = dbgo.tile([128, d], F16, tag="o")
                            nc.vector.tensor_scalar_mul(o_t[:], sup_sb[:, b, :],
                                                        scale0[:, b:b + 1])
                            nc.sync.dma_start(out_d[b * 128:(b + 1) * 128, :],
                                              o_t[:])

                # ---- main loop: masked row softmax + SpMM, online over sweeps ----
                with (
                    tc.tile_pool(name="adjp", bufs=2) as adjp,
                    tc.tile_pool(name="bitsp", bufs=2) as bitsp,
                    tc.tile_pool(name="tp", bufs=2) as tp,
                    tc.tile_pool(name="ttp", bufs=2) as ttp,
                    tc.tile_pool(name="accp", bufs=2) as accp,
                    tc.tile_pool(name="outp", bufs=2) as outp,
                    tc.tile_pool(name="epsp", bufs=2, space="PSUM") as epsp,
                    tc.tile_pool(name="spsp", bufs=2, space="PSUM") as spsp,
                    tc.tile_pool(name="ttpsp", bufs=2, space="PSUM") as ttpsp,
                ):
                    for b in ([] if dbg_skip_main else range(rb)):
                        oacc = accp.tile([128, d], F32, tag="oacc")
                        zacc = vecp.tile([128, 1], F32, tag="zacc")
                        rmrun = None
                        for q in range(nsweep):
                            # unpack adjacency bits -> f16 {0,1} on DVE
                            bits_t = bitsp.tile([128, jsweep], U8, tag="bits")
                            bsl = slice(q * (jsweep // 8),
                                        (q + 1) * (jsweep // 8))
                            for k in range(8):
                                nc.vector.tensor_scalar(
                                    out=bits_t[:, k::8],
                                    in0=adjp_sb[:, b, bsl],
                                    scalar1=k, scalar2=1,
                                    op0=ALU.logical_shift_right,
                                    op1=ALU.bitwise_and)
                            adj_t = adjp.tile([128, jsweep], F16, tag="adj")
                            nc.vector.tensor_copy(out=adj_t[:], in_=bits_t[:])
                            # PSUM seeded with 1000*adj (identity matmul),
                            # e accumulates on top: kept cols sit ~1000 above
                            # masked ones, so exp(psum - rowmax) masks exactly
                            # whole sweep in one 2-bank psum tile: one
                            # negate-fused reduce and one exp for the sweep
                            eps = epsp.tile([128, jsweep], F32, tag="eps")
                            for j in range(jt):
                                joff = q * jsweep + j * 512
                                sl = slice(j * 512, (j + 1) * 512)
                                nc.tensor.matmul(
                                    eps[:, sl], idk_sb[:],
                                    adj_t[:, sl], start=True, stop=False)
                                for c in range(kc):
                                    nc.tensor.matmul(
                                        eps[:, sl],
                                        hT_sb[:, c, b * 128:(b + 1) * 128],
                                        hT_sb[:, c, joff:joff + 512],
                                        start=False, stop=(c == kc - 1))
                            nrmq = vecp.tile([128, 1], F32, tag="nrmq")
                            nc.vector.tensor_reduce(nrmq[:], eps[:], axis=AX.X,
                                                    op=ALU.max, negate=True)
                            t_t = tp.tile([128, jsweep], F16, tag="t")
                            zq = vecp.tile([128, 1], F32, tag="zq")
                            nc.scalar.activation(t_t[:], eps[:], ACTF.Exp,
                                                 bias=nrmq[:], accum_out=zq[:])
                            # transpose t 128-chunks, SpMM against support
                            S = spsp.tile([128, d], F32, tag="S")
                            for g in range(jc // gw):
                                ttps = ttpsp.tile([128, 128 * gw], F16, tag="ttps")
                                for u in range(gw):
                                    ch = g * gw + u
                                    nc.tensor.transpose(
                                        ttps[:, u * 128:(u + 1) * 128],
                                        t_t[:, ch * 128:(ch + 1) * 128], id_sb[:])
                                tt_sb = ttp.tile([128, 128 * gw], F16, tag="tt")
                                nc.vector.tensor_copy(out=tt_sb[:], in_=ttps[:])
                                for u in range(gw):
                                    jchunk = q * jc + g * gw + u
                                    nc.tensor.matmul(
                                        S[:], tt_sb[:, u * 128:(u + 1) * 128],
                                        sup_sb[:, jchunk, :],
                                        start=(g == 0 and u == 0),
                                        stop=(g == jc // gw - 1 and u == gw - 1))
                            if q == 0:
                                nc.vector.tensor_copy(out=oacc[:], in_=S[:])
                                nc.vector.tensor_copy(out=zacc[:], in_=zq[:])
                                rmrun = nrmq
                            else:
                                rmnew = vecp.tile([128, 1], F32, tag="rmnew")
                                nc.vector.tensor_tensor(rmnew[:], rmrun[:], nrmq[:],
                                                        ALU.min)
                                dold = vecp.tile([128, 1], F32, tag="dold")
                                nc.vector.tensor_tensor(dold[:], rmnew[:], rmrun[:],
                                                        ALU.subtract)
                                dq = vecp.tile([128, 1], F32, tag="dq")
                                nc.vector.tensor_tensor(dq[:], rmnew[:], nrmq[:],
                                                        ALU.subtract)
                                cold = vecp.tile([128, 1], F32, tag="cold")
                                nc.scalar.activation(cold[:], dold[:], ACTF.Exp)
                                cq = vecp.tile([128, 1], F32, tag="cq")
                                nc.scalar.activation(cq[:], dq[:], ACTF.Exp)
                                nc.vector.tensor_scalar_mul(oacc[:], oacc[:], cold[:])
                                nc.vector.scalar_tensor_tensor(
                                    out=oacc[:], in0=S[:], scalar=cq[:],
                                    in1=oacc[:], op0=ALU.mult, op1=ALU.add)
                                nc.vector.tensor_scalar_mul(zacc[:], zacc[:], cold[:])
                                nc.vector.scalar_tensor_tensor(
                                    out=zacc[:], in0=zq[:], scalar=cq[:],
                                    in1=zacc[:], op0=ALU.mult, op1=ALU.add)
                                rmrun = rmnew
                        zi = vecp.tile([128, 1], F32, tag="zi")
                        nc.vector.reciprocal(zi[:], zacc[:])
                        scb = vecp.tile([128, 1], F32, tag="scb")
                        nc.vector.tensor_tensor(scb[:], zi[:], scale0[:, b:b + 1],
                                                ALU.mult)
                        o_t = outp.tile([128, d], F16, tag="o")
                        nc.vector.scalar_tensor_tensor(
                            out=o_t[:], in0=oacc[:], scalar=scb[:],
                            in1=bias_sb[:], op0=ALU.mult, op1=ALU.add)
                        nc.sync.dma_start(out_d[b * 128:(b + 1) * 128, :], o_t[:])

    nc.finalize()
    return nc


def make_in_maps(x, adj, weight, bias, node_w, edge_w, n=N, d=D, ncores=NCORES):
    loc = n // ncores
    kc = d // 128
    xt = np.ascontiguousarray(x.T.astype(np.float16)).reshape(kc, 128, n)
    ew = np.ascontiguousarray(edge_w.astype(np.float16)).reshape(kc, 128, d)
    wt = np.ascontiguousarray(weight.astype(np.float16)).reshape(kc, 128, d)
    nw = np.ascontiguousarray(node_w.astype(np.float16)).reshape(kc, 128, 1)
    biasr = np.ascontiguousarray(bias.astype(np.float32)[None, :])
    ident = np.eye(128, dtype=np.float16)
    adjb = adj > 0
    in_maps = []
    for c in range(ncores):
        sh = c * loc
        xt_c = np.ascontiguousarray(np.roll(xt, -sh, axis=2))
        adjp_c = np.packbits(np.roll(adjb[sh:sh + loc], -sh, axis=1),
                             axis=1, bitorder="little")
        in_maps.append({"xt": xt_c, "adjp": adjp_c, "ew": ew, "wt": wt,
                        "nw": nw, "biasr": biasr, "ident": ident})
    return in_maps


_CACHE = {}


def kernel(x, adj, weight, bias, node_w, edge_w):
    x = np.asarray(x)
    adj = np.asarray(adj)
    weight = np.asarray(weight)
    bias = np.asarray(bias)
    node_w = np.asarray(node_w)
    edge_w = np.asarray(edge_w)
    assert x.shape == (N, D) and adj.shape == (N, N)
    if "nc" not in _CACHE:
        _CACHE["nc"] = build_program()
    nc = _CACHE["nc"]
    in_maps = make_in_maps(x, adj, weight, bias, node_w, edge_w)
    res = run_bass_kernel_spmd(nc, in_maps, list(range(NCORES)))
    out = np.concatenate([res.results[c]["out"] for c in range(NCORES)], axis=0)
    return np.ascontiguousarray(out.astype(np.float32))



# revision 25
# speedup vs baseline: 1.2811x; 1.2811x over previous
"""Trainium2 Bass kernel for DualGraphConvolution.

reference math (N=8192, D=512):
    node_att = softmax(x @ node_w, axis=0)            # [N, 1]
    h        = x @ edge_w                             # [N, D]
    e        = h @ h.T ; masked where adj <= 0        # [N, N]
    edge_att = softmax(e, axis=1)                     # [N, N]
    out      = (adj * node_att * edge_att) @ (x @ weight) + bias

Distribution: row-shard the N dimension over 8 NeuronCores (1024 rows each).
Each core receives a *rotated* copy of the inputs (its own rows first) so the
SPMD program only ever uses static indices; column order of the j-contraction
is irrelevant because it is summed over.

Per core the kernel computes, for its 1024 rows r:
    m[r, j] = e[r, j] + (adj[r, j] - 1) * 1000        # masked-out cols pushed
                                                      # ~-1000 below the row max
    rowmax, t = exp(m - rowmax), Z = sum t            # online softmax over 4
                                                      # column sweeps of 2048
    O[r, :] = t @ support                             # support = x @ weight
    out = O * (exp(p_r) / (sum_k exp(p_k)) / Z) + bias  # p = x @ node_w

Matmul operands are fp16 (PE runs them at full rate; ~1e-3 relative error),
all accumulation/softmax state is fp32.
"""

import numpy as np

import concourse.bass as bass
import concourse.mybir as mybir
import concourse.tile as tile
from concourse import bacc
from concourse.bass_utils import run_bass_kernel_spmd

F16 = mybir.dt.float16
F32 = mybir.dt.float32
U8 = mybir.dt.uint8
ALU = mybir.AluOpType
ACTF = mybir.ActivationFunctionType
AX = mybir.AxisListType

N = 8192
D = 512
NCORES = 8
JSWEEP = 1024  # columns per online-softmax sweep
NEG_INIT = -3.0e38
MASK_SHIFT = 1000.0  # adj==0 columns get e - 1000: far below row max, exp -> 0


def build_program(n=N, d=D, ncores=NCORES, jsweep=JSWEEP,
                  dbg_skip_main=False, dbg_skip_nodestats=False,
                  dbg_main_upto=None, body_reps=1):
    loc = n // ncores          # rows owned by this core
    rb = loc // 128            # 128-row blocks per core
    kc = d // 128              # contraction chunks of 128
    nsweep = n // jsweep       # online-softmax sweeps
    jt = jsweep // 512         # 512-col j tiles per sweep
    jc = jsweep // 128         # 128-col j chunks per sweep
    rchunks = n // 128
    gw = 8 if jc % 8 == 0 else 4
    assert jc % gw == 0

    nc = bacc.Bacc("TRN2", target_bir_lowering=False, debug=False,
                   num_devices=ncores)

    xt_d = nc.dram_tensor("xt", [kc, 128, n], F16, kind="ExternalInput")
    adj_d = nc.dram_tensor("adj", [loc, n], F16, kind="ExternalInput")
    ew_d = nc.dram_tensor("ew", [kc, 128, d], F16, kind="ExternalInput")
    wt_d = nc.dram_tensor("wt", [kc, 128, d], F16, kind="ExternalInput")
    nw_d = nc.dram_tensor("nw", [kc, 128, 1], F16, kind="ExternalInput")
    bias_d = nc.dram_tensor("biasr", [1, d], F32, kind="ExternalInput")
    id_d = nc.dram_tensor("ident", [128, 128], F16, kind="ExternalInput")
    out_d = nc.dram_tensor("out", [loc, d], F16, kind="ExternalOutput")

    with tile.TileContext(nc) as tc:
        for _rep in range(body_reps):
            with (
                tc.tile_pool(name="const", bufs=1) as constp,
                tc.tile_pool(name="big", bufs=1) as bigp,
                tc.tile_pool(name="vec", bufs=6) as vecp,
            ):
                ew_sb = constp.tile([128, kc, d], F16)
                nc.sync.dma_start(ew_sb[:], ew_d.rearrange("c p f -> p c f"))
                wt_sb = constp.tile([128, kc, d], F16)
                nc.sync.dma_start(wt_sb[:], wt_d.rearrange("c p f -> p c f"))
                nw_sb = constp.tile([128, kc, 1], F16)
                nc.sync.dma_start(nw_sb[:], nw_d.rearrange("c p f -> p c f"))
                bias1_sb = constp.tile([1, d], F32)
                nc.sync.dma_start(bias1_sb[:], bias_d[:])
                id_sb = constp.tile([128, 128], F16)
                nc.sync.dma_start(id_sb[:], id_d[:])

                hT_sb = bigp.tile([128, kc, n], F16)   # h[r, dd] at [dd%128, dd//128, r]
                sup_sb = bigp.tile([128, rchunks, d], F16)  # support[rc*128+p, f]
                pzp = bigp.tile([1, n // 512], F32)    # per-r-tile sums of exp(p)
                ploc_sb = bigp.tile([128, rb], F32)    # exp(p) for local rows
                ones_row = constp.tile([1, 128], F32)
                nc.vector.memset(ones_row[:], 1.0)
                # sweep-0 mask seed: e + 1000*adj - 1000 keeps kept cols at
                # raw-e scale while pushing masked cols (incl. the |h|^2
                # diagonal) ~1000 below; later sweeps are pure off-diagonal
                # and need no seed
                idk_sb = constp.tile([128, 128], F16)
                nc.vector.tensor_scalar_mul(idk_sb[:], id_sb[:], MASK_SHIFT)
                negones = constp.tile([128, 512], F16)
                nc.vector.memset(negones[:], -1.0)

                # ---- phase 0 (replicated): hT, support, p for all rows ----
                xt_view = xt_d.rearrange("c p r -> p c r")
                with (
                    tc.tile_pool(name="ph0", bufs=3) as ph0p,
                    tc.tile_pool(name="ph0ps", bufs=2, space="PSUM") as ph0ps,
                    tc.tile_pool(name="ph0ps1", bufs=1, space="PSUM") as ph0ps1,
                ):
                    for rt in range(n // 512):
                        xt_t = ph0p.tile([128, kc, 512], F16, tag="xt")
                        nc.sync.dma_start(
                            xt_t[:], xt_view[:, :, rt * 512:(rt + 1) * 512])
                        for dcp in range(kc // 2):
                            # two 512-wide groups into one 2-bank psum tile,
                            # one strided copy writes both hT d-chunks
                            hps = ph0ps.tile([128, 1024], F32, tag="hps")
                            for half in range(2):
                                dc = dcp * 2 + half
                                for c in range(kc):
                                    nc.tensor.matmul(
                                        hps[:, half * 512:(half + 1) * 512],
                                        ew_sb[:, c, dc * 128:(dc + 1) * 128],
                                        xt_t[:, c, :], start=(c == 0),
                                        stop=(c == kc - 1))
                            nc.vector.tensor_copy(
                                out=hT_sb[:, dcp * 2:dcp * 2 + 2,
                                          rt * 512:(rt + 1) * 512],
                                in_=hps[:].rearrange("p (h f) -> p h f", h=2))
                        for rs in range(4):
                            rch = rt * 4 + rs
                            sps = ph0ps.tile([128, d], F32, tag="sps")
                            for c in range(kc):
                                nc.tensor.matmul(
                                    sps[:], xt_t[:, c, rs * 128:(rs + 1) * 128],
                                    wt_sb[:, c, :], start=(c == 0), stop=(c == kc - 1))
                            nc.vector.tensor_copy(out=sup_sb[:, rch, :],
                                                  in_=sps[:])
                        # p slice [1, 512] via nw as the 1-col stationary
                        pps = ph0ps1.tile([1, 512], F32, tag="pps")
                        for c in range(kc):
                            nc.tensor.matmul(
                                pps[:], nw_sb[:, c, :], xt_t[:, c, :],
                                start=(c == 0), stop=(c == kc - 1))
                        pe_t = ph0p.tile([1, 512], F32, tag="pe")
                        nc.scalar.activation(pe_t[:], pps[:], ACTF.Exp,
                                             accum_out=pzp[:, rt:rt + 1])
                        if rt * 512 < loc:
                            # local rows: spread exp(p) across partitions via
                            # K=1 matmuls with the [1,128] slice stationary
                            for i in range(4):
                                b0 = rt * 4 + i
                                if b0 >= rb:
                                    break
                                tp_ps = ph0ps1.tile([128, 1], F32, tag="ptp")
                                nc.tensor.matmul(
                                    tp_ps[:], pe_t[:, i * 128:(i + 1) * 128],
                                    ones_row[:, 0:1])
                                nc.vector.tensor_copy(
                                    out=ploc_sb[:, b0:b0 + 1], in_=tp_ps[:])

                # ---- bias broadcast [1,d] -> [128,d] via K=1 matmul ----
                bias_sb = bigp.tile([128, d], F32)
                with tc.tile_pool(name="bbr", bufs=1, space="PSUM") as bbp:
                    bps = bbp.tile([128, d], F32, tag="bps")
                    nc.tensor.matmul(bps[:], ones_row[:], bias1_sb[:])
                    nc.vector.tensor_copy(out=bias_sb[:], in_=bps[:])

                # ---- node attention: scale0 = exp(p_loc) / sum(exp(p)) ----
                scale0 = bigp.tile([128, rb], F32)
                if dbg_skip_nodestats:
                    nc.vector.memset(scale0[:], 1.0)
                else:
                    pz = vecp.tile([1, 1], F32, tag="pz")
                    nc.vector.reduce_sum(pz[:], pzp[:], axis=AX.X)
                    pzi = vecp.tile([1, 1], F32, tag="pzi")
                    nc.vector.reciprocal(pzi[:], pz[:])
                    with tc.tile_pool(name="nps", bufs=2, space="PSUM") as npsp:
                        # broadcast 1/pz to all partitions via K=1 matmul
                        pzb_ps = npsp.tile([128, 1], F32, tag="pzbps")
                        nc.tensor.matmul(pzb_ps[:], ones_row[:], pzi[:])
                        pzb = vecp.tile([128, 1], F32, tag="pzb")
                        nc.vector.tensor_copy(out=pzb[:], in_=pzb_ps[:])
                    nc.vector.tensor_scalar_mul(scale0[:], ploc_sb[:], pzb[:])

                if dbg_skip_main:
                    with tc.tile_pool(name="dbgo", bufs=2) as dbgo:
                        for b in range(rb):
                            o_t = dbgo.tile([128, d], F16, tag="o")
                            nc.vector.tensor_scalar_mul(o_t[:], sup_sb[:, b, :],
                                                        scale0[:, b:b + 1])
                            nc.sync.dma_start(out_d[b * 128:(b + 1) * 128, :],
                                              o_t[:])

                # ---- main loop: masked row softmax + SpMM, online over sweeps ----
                with (
                    tc.tile_pool(name="adjp", bufs=2) as adjp,
                    tc.tile_pool(name="tp", bufs=2) as tp,
                    tc.tile_pool(name="ttp", bufs=2) as ttp,
                    tc.tile_pool(name="accp", bufs=2) as accp,
                    tc.tile_pool(name="outp", bufs=2) as outp,
                    tc.tile_pool(name="epsp", bufs=2, space="PSUM") as epsp,
                    tc.tile_pool(name="spsp", bufs=2, space="PSUM") as spsp,
                    tc.tile_pool(name="ttpsp", bufs=2, space="PSUM") as ttpsp,
                ):
                    for b in ([] if dbg_skip_main else range(rb)):
                        oacc = accp.tile([128, d], F32, tag="oacc")
                        zacc = vecp.tile([128, 1], F32, tag="zacc")
                        rmrun = None
                        for q in range(nsweep):
                            adj_t = adjp.tile([128, jsweep], F16, tag="adj")
                            nc.sync.dma_start(
                                adj_t[:],
                                adj_d[b * 128:(b + 1) * 128,
                                      q * jsweep:(q + 1) * jsweep])
                            # raw e for the whole sweep in one 2-bank psum
                            # tile; masking happens post-exp (t *= adj) so no
                            # seed matmul is needed
                            eps = epsp.tile([128, jsweep], F32, tag="eps")
                            for j in range(jt):
                                joff = q * jsweep + j * 512
                                sl = slice(j * 512, (j + 1) * 512)
                                if q == 0:
                                    nc.tensor.matmul(
                                        eps[:, sl], idk_sb[:], adj_t[:, sl],
                                        start=True, stop=False)
                                    nc.tensor.matmul(
                                        eps[:, sl], idk_sb[:], negones[:],
                                        start=False, stop=False)
                                for c in range(kc):
                                    nc.tensor.matmul(
                                        eps[:, sl],
                                        hT_sb[:, c, b * 128:(b + 1) * 128],
                                        hT_sb[:, c, joff:joff + 512],
                                        start=(q > 0 and c == 0),
                                        stop=(c == kc - 1))
                            nrmq = vecp.tile([128, 1], F32, tag="nrmq")
                            nc.vector.tensor_reduce(nrmq[:], eps[:], axis=AX.X,
                                                    op=ALU.max, negate=True)
                            traw = tp.tile([128, jsweep], F16, tag="traw")
                            nc.scalar.activation(traw[:], eps[:], ACTF.Exp,
                                                 bias=nrmq[:])
                            # mask + Z in one DVE pass: t = traw*adj,
                            # zq = sum(t)
                            t_t = tp.tile([128, jsweep], F16, tag="t")
                            zq = vecp.tile([128, 1], F32, tag="zq")
                            nc.vector.scalar_tensor_tensor(
                                out=t_t[:], in0=traw[:], scalar=1.0,
                                in1=adj_t[:], op0=ALU.mult, op1=ALU.mult,
                                accum_out=zq[:])
                            # transpose t 128-chunks, SpMM against support
                            S = spsp.tile([128, d], F32, tag="S")
                            for g in range(jc // gw):
                                ttps = ttpsp.tile([128, 128 * gw], F16, tag="ttps")
                                for u in range(gw):
                                    ch = g * gw + u
                                    nc.tensor.transpose(
                                        ttps[:, u * 128:(u + 1) * 128],
                                        t_t[:, ch * 128:(ch + 1) * 128], id_sb[:])
                                tt_sb = ttp.tile([128, 128 * gw], F16, tag="tt")
                                nc.vector.tensor_copy(out=tt_sb[:], in_=ttps[:])
                                for u in range(gw):
                                    jchunk = q * jc + g * gw + u
                                    nc.tensor.matmul(
                                        S[:], tt_sb[:, u * 128:(u + 1) * 128],
                                        sup_sb[:, jchunk, :],
                                        start=(g == 0 and u == 0),
                                        stop=(g == jc // gw - 1 and u == gw - 1))
                            if q == 0:
                                nc.vector.tensor_copy(out=oacc[:], in_=S[:])
                                nc.vector.tensor_copy(out=zacc[:], in_=zq[:])
                                rmrun = nrmq
                            else:
                                rmnew = vecp.tile([128, 1], F32, tag="rmnew")
                                nc.vector.tensor_tensor(rmnew[:], rmrun[:], nrmq[:],
                                                        ALU.min)
                                dold = vecp.tile([128, 1], F32, tag="dold")
                                nc.vector.tensor_tensor(dold[:], rmnew[:], rmrun[:],
                                                        ALU.subtract)
                                dq = vecp.tile([128, 1], F32, tag="dq")
                                nc.vector.tensor_tensor(dq[:], rmnew[:], nrmq[:],
                                                        ALU.subtract)
                                cold = vecp.tile([128, 1], F32, tag="cold")
                                nc.scalar.activation(cold[:], dold[:], ACTF.Exp)
                                cq = vecp.tile([128, 1], F32, tag="cq")
                                nc.scalar.activation(cq[:], dq[:], ACTF.Exp)
                                nc.vector.tensor_scalar_mul(oacc[:], oacc[:], cold[:])
                                nc.vector.scalar_tensor_tensor(
                                    out=oacc[:], in0=S[:], scalar=cq[:],
                                    in1=oacc[:], op0=ALU.mult, op1=ALU.add)
                                nc.vector.tensor_scalar_mul(zacc[:], zacc[:], cold[:])
                                nc.vector.scalar_tensor_tensor(
                                    out=zacc[:], in0=zq[:], scalar=cq[:],
                                    in1=zacc[:], op0=ALU.mult, op1=ALU.add)
                                rmrun = rmnew
                        zi = vecp.tile([128, 1], F32, tag="zi")
                        nc.vector.reciprocal(zi[:], zacc[:])
                        scb = vecp.tile([128, 1], F32, tag="scb")
                        nc.vector.tensor_tensor(scb[:], zi[:], scale0[:, b:b + 1],
                                                ALU.mult)
                        o_t = outp.tile([128, d], F16, tag="o")
                        nc.vector.scalar_tensor_tensor(
                            out=o_t[:], in0=oacc[:], scalar=scb[:],
                            in1=bias_sb[:], op0=ALU.mult, op1=ALU.add)
                        nc.sync.dma_start(out_d[b * 128:(b + 1) * 128, :], o_t[:])

    nc.finalize()
    return nc


def make_in_maps(x, adj, weight, bias, node_w, edge_w, n=N, d=D, ncores=NCORES):
    loc = n // ncores
    kc = d // 128
    xt = np.ascontiguousarray(x.T.astype(np.float16)).reshape(kc, 128, n)
    ew = np.ascontiguousarray(edge_w.astype(np.float16)).reshape(kc, 128, d)
    wt = np.ascontiguousarray(weight.astype(np.float16)).reshape(kc, 128, d)
    nw = np.ascontiguousarray(node_w.astype(np.float16)).reshape(kc, 128, 1)
    biasr = np.ascontiguousarray(bias.astype(np.float32)[None, :])
    ident = np.eye(128, dtype=np.float16)
    adj16 = adj.astype(np.float16)
    in_maps = []
    for c in range(ncores):
        sh = c * loc
        xt_c = np.ascontiguousarray(np.roll(xt, -sh, axis=2))
        adj_c = np.ascontiguousarray(np.roll(adj16[sh:sh + loc], -sh, axis=1))
        in_maps.append({"xt": xt_c, "adj": adj_c, "ew": ew, "wt": wt,
                        "nw": nw, "biasr": biasr, "ident": ident})
    return in_maps


_CACHE = {}


def kernel(x, adj, weight, bias, node_w, edge_w):
    x = np.asarray(x)
    adj = np.asarray(adj)
    weight = np.asarray(weight)
    bias = np.asarray(bias)
    node_w = np.asarray(node_w)
    edge_w = np.asarray(edge_w)
    assert x.shape == (N, D) and adj.shape == (N, N)
    if "nc" not in _CACHE:
        _CACHE["nc"] = build_program()
    nc = _CACHE["nc"]
    in_maps = make_in_maps(x, adj, weight, bias, node_w, edge_w)
    res = run_bass_kernel_spmd(nc, in_maps, list(range(NCORES)))
    out = np.concatenate([res.results[c]["out"] for c in range(NCORES)], axis=0)
    return np.ascontiguousarray(out.astype(np.float32))

